# revision 1
# baseline (speedup 1.0000x reference)
"""AdaptiveRotatedConv2d on 8 TRN2 NeuronCores (data-parallel over batch).

Per core (2 samples):
  Stage A: rotated weights rw[b,p,cin,cout] = sum_{n,q} rm[b,n,p,q] * W[n,cout,cin,q]
           via TensorE matmuls: lhsT = rmt [36, 18] (stationary),
           rhs = wf [36, cin*256+cout] (moving, 512-chunks), PSUM [18, 512].
           Cast-copy to bf16, bounce through DRAM scratch [b*9+p, cin, cout].
  Stage B: conv as 9-tap shifted matmuls over a zero-padded [128, 66, 66]
           SBUF image; lhsT = rw tiles [cin=128, cout=128] read back from
           scratch (the DRAM bounce performs the (b,p)->cin transpose);
           accumulate 18 matmuls per PSUM group [cout=128, 8 rows x 64].
"""

from contextlib import ExitStack

import ml_dtypes
import numpy as np

import concourse.bass as bass
import concourse.tile as tile
from concourse import bacc, mybir
from concourse.bass_utils import run_bass_kernel_spmd

B, N, COUT, CIN, H, W = 16, 4, 256, 256, 64, 64
NCORES = 8
BPC = B // NCORES            # samples per core
NTAP = 9
KA = N * NTAP                # 36  stage-A contraction (n, q)
MA = BPC * NTAP              # 18  stage-A output rows (b_local, p)
HP, WP = H + 2, W + 2        # 66x66 padded image
M_TOT = CIN * COUT           # 65536
ROWS_PER_CHUNK = 8           # output rows per PSUM group (N = 8*64 = 512)
NCHUNK = H // ROWS_PER_CHUNK # 8
CT = CIN // 128              # cin tiles (2)
OT = COUT // 128             # cout tiles (2)

BF16 = mybir.dt.bfloat16
F32 = mybir.dt.float32


def _build_graph():
    nc = bacc.Bacc(None, target_bir_lowering=False)

    x_d = nc.dram_tensor("x", [BPC, CIN, H, W], BF16, kind="ExternalInput")
    wf_d = nc.dram_tensor("wf", [KA, M_TOT], BF16, kind="ExternalInput")
    rmt_d = nc.dram_tensor("rmt", [KA, MA], BF16, kind="ExternalInput")
    out_d = nc.dram_tensor("out", [BPC, COUT, H, W], F32, kind="ExternalOutput")

    with tile.TileContext(nc) as tc, ExitStack() as ctx:
        const_pool = ctx.enter_context(tc.tile_pool(name="const", bufs=1))
        wf_pool = ctx.enter_context(tc.tile_pool(name="wfin", bufs=4))
        rwst_pool = ctx.enter_context(tc.tile_pool(name="rwst", bufs=2))
        xpad_pool = ctx.enter_context(tc.tile_pool(name="xpad", bufs=1))
        rwt_pool = ctx.enter_context(tc.tile_pool(name="rwt", bufs=4))
        out_pool = ctx.enter_context(tc.tile_pool(name="outs", bufs=4))
        scr_pool = ctx.enter_context(tc.tile_pool(name="scr", bufs=1, space="DRAM"))

        scr = scr_pool.tile([MA, CIN, COUT], BF16)

        # ---- stationary rotation-mix matrix ----
        rmt_sb = const_pool.tile([KA, MA], BF16)
        nc.sync.dma_start(rmt_sb[:], rmt_d[:])

        # ---- padded input images (zero border), overlapped with stage A ----
        xpads = []
        for b in range(BPC):
            row = []
            for ct in range(CT):
                xp = xpad_pool.tile([128, HP, WP], BF16, tag=f"xp{b}{ct}")
                nc.gpsimd.memset(xp[:], 0.0)
                nc.sync.dma_start(
                    xp[:, 1 : H + 1, 1 : W + 1],
                    x_d[b, ct * 128 : (ct + 1) * 128],
                )
                row.append(xp)
            xpads.append(row)

        # ---- Stage A: rw = rmt.T @ wf, chunked over m' = cin*COUT+cout ----
        # group 4 x 512-chunks into one PSUM tile (4 banks), one cast-copy, one DMA
        GROUP = 4 * 512
        NGRP = M_TOT // GROUP  # 32
        scr_flat = scr[:].rearrange("m c o -> m (c o)")
        with tc.tile_pool(name="psa", bufs=2, space="PSUM") as psa_pool:
            for g in range(NGRP):
                wf_sb = wf_pool.tile([KA, GROUP], BF16)
                nc.sync.dma_start(wf_sb[:], wf_d[:, g * GROUP : (g + 1) * GROUP])
                ps = psa_pool.tile([MA, GROUP], F32)
                for j in range(4):
                    nc.tensor.matmul(
                        ps[:, j * 512 : (j + 1) * 512],
                        rmt_sb[:],
                        wf_sb[:, j * 512 : (j + 1) * 512],
                        start=True,
                        stop=True,
                    )
                rw_sb = rwst_pool.tile([MA, GROUP], BF16)
                nc.vector.tensor_copy(rw_sb[:], ps[:])
                nc.sync.dma_start(scr_flat[:, g * GROUP : (g + 1) * GROUP], rw_sb[:])

        # ---- Stage B: per-sample grouped conv ----
        with tc.tile_pool(name="psb", bufs=4, space="PSUM") as psb_pool:
            for b in range(BPC):
                for ot in range(OT):
                    # load rw tiles [cin=128, (p, cout=128)] for both cin halves
                    rwts = []
                    for ct in range(CT):
                        rwt = rwt_pool.tile([128, NTAP, 128], BF16, tag=f"rwt{ct}")
                        src = scr[
                            b * NTAP : (b + 1) * NTAP,
                            ct * 128 : (ct + 1) * 128,
                            ot * 128 : (ot + 1) * 128,
                        ].rearrange("p c o -> c p o")
                        nc.sync.dma_start(rwt[:], src)
                        rwts.append(rwt)

                    for yc in range(NCHUNK):
                        ps = psb_pool.tile([128, ROWS_PER_CHUNK, W], F32)
                        first = True
                        for ct in range(CT):
                            xp = xpads[b][ct]
                            for p in range(NTAP):
                                py, px = p // 3, p % 3
                                y0 = yc * ROWS_PER_CHUNK + py
                                nc.tensor.matmul(
                                    ps[:],
                                    rwts[ct][:, p, :],
                                    xp[:, y0 : y0 + ROWS_PER_CHUNK, px : px + W],
                                    start=first,
                                    stop=(ct == CT - 1 and p == NTAP - 1),
                                )
                                first = False
                        ot_sb = out_pool.tile([128, ROWS_PER_CHUNK, W], F32)
                        nc.vector.tensor_copy(ot_sb[:], ps[:])
                        nc.sync.dma_start(
                            out_d[
                                b,
                                ot * 128 : (ot + 1) * 128,
                                yc * ROWS_PER_CHUNK : (yc + 1) * ROWS_PER_CHUNK,
                            ],
                            ot_sb[:],
                        )

    nc.compile()
    return nc


_NC = None


def _get_nc():
    global _NC
    if _NC is None:
        _NC = _build_graph()
    return _NC


def _rot_mats_np(thetas):
    """thetas: [M] -> [M, 9, 9], numpy port of the reference builder."""
    thetas = np.asarray(thetas, np.float32)
    xc = np.cos(thetas)
    ys = np.sin(thetas)
    a = xc - ys
    b = xc * ys
    c = xc + ys
    z = np.zeros_like(xc)
    o = np.ones_like(xc)

    def mat(rows):
        return np.stack([np.stack(r, axis=-1) for r in rows], axis=-2)

    pos = mat([
        [a, 1 - a, z, z, z, z, z, z, z],
        [z, xc - b, b, z, 1 - c + b, ys - b, z, z, z],
        [z, z, a, z, z, 1 - a, z, z, z],
        [b, ys - b, z, xc - b, 1 - c + b, z, z, z, z],
        [z, z, z, z, o, z, z, z, z],
        [z, z, z, z, 1 - c + b, xc - b, z, ys - b, b],
        [z, z, z, 1 - a, z, z, a, z, z],
        [z, z, z, ys - b, 1 - c + b, z, b, xc - b, z],
        [z, z, z, z, z, z, z, 1 - a, a],
    ])
    neg = mat([
        [c, z, z, 1 - c, z, z, z, z, z],
        [-b, xc + b, z, b - ys, 1 - a - b, z, z, z, z],
        [z, 1 - c, c, z, z, z, z, z, z],
        [z, z, z, xc + b, 1 - a - b, z, -b, b - ys, z],
        [z, z, z, z, o, z, z, z, z],
        [z, b - ys, -b, z, 1 - a - b, xc + b, z, z, z],
        [z, z, z, z, z, z, c, 1 - c, z],
        [z, z, z, z, 1 - a - b, b - ys, z, xc + b, -b],
        [z, z, z, z, z, 1 - c, z, z, c],
    ])
    m = (thetas >= 0).astype(np.float32)[:, None, None]
    return m * pos + (1 - m) * neg


def _prep_inputs(x, alphas, angles, weight):
    x = np.asarray(x, np.float32)
    alphas = np.asarray(alphas, np.float32)
    angles = np.asarray(angles, np.float32)
    weight = np.asarray(weight, np.float32)

    rm = _rot_mats_np(angles.reshape(-1)).reshape(B, N, NTAP, NTAP)
    rm = rm * alphas[:, :, None, None]          # [b, n, p, q]
    # lhsT rows (n, q) = n*9+q ; cols (b, p) = b*9+p
    rmt = rm.transpose(1, 3, 0, 2).reshape(KA, B, NTAP)

    # wf rows (n, qy, qx) ; cols cin*COUT + cout
    wf = weight.transpose(0, 3, 4, 2, 1).reshape(KA, M_TOT)

    x_bf = x.astype(ml_dtypes.bfloat16)
    wf_bf = np.ascontiguousarray(wf).astype(ml_dtypes.bfloat16)

    in_maps = []
    for i in range(NCORES):
        in_maps.append({
            "x": np.ascontiguousarray(x_bf[i * BPC : (i + 1) * BPC]),
            "wf": wf_bf,
            "rmt": np.ascontiguousarray(
                rmt[:, i * BPC : (i + 1) * BPC].reshape(KA, MA)
            ).astype(ml_dtypes.bfloat16),
        })
    return in_maps


def _run(inputs, trace=False, **kw):
    nc = _get_nc()
    in_maps = _prep_inputs(**inputs)
    br = run_bass_kernel_spmd(nc, in_maps, core_ids=list(range(NCORES)),
                              trace=trace, **kw)
    out = np.concatenate([r["out"] for r in br.results], axis=0)
    return out, br


def kernel(x, alphas, angles, weight):
    out, _ = _run(dict(x=x, alphas=alphas, angles=angles, weight=weight))
    return out


if __name__ == "__main__":
    rng = np.random.default_rng(0)
    x = rng.standard_normal((B, CIN, H, W), np.float32)
    alphas = rng.random((B, N), np.float32)
    angles = (rng.standard_normal((B, N), np.float32) * 0.5).astype(np.float32)
    weight = rng.standard_normal((N, COUT, CIN, 3, 3), np.float32) * np.sqrt(
        2.0 / (COUT * 9)
    ).astype(np.float32)
    out = kernel(x=x, alphas=alphas, angles=angles, weight=weight)
    print(out.shape, out.dtype, np.abs(out).mean())


# revision 5
# speedup vs baseline: 1.1047x; 1.1047x over previous
"""AdaptiveRotatedConv2d on 8 TRN2 NeuronCores (data-parallel over batch).

Per core (2 samples):
  Stage A: rotated weights rw[b,p,cin,cout] = sum_{n,q} rm[b,n,p,q] * W[n,cout,cin,q]
           via TensorE matmuls: lhsT = rmt [36, 32] (stationary, cols 18..31 zero),
           rhs = wf [36, m'] chunks (moving, m' = cin*COUT+cout), packed 4x into
           PSUM partition col-groups via tile_position -> PSUM [128, 2048] holds
           16 chunks; one DVE cast-copy per tile; bounce through DRAM scratch
           [b*9+p, cin, cout] (the bounce performs the (b,p)->cin transpose).
  Stage B: conv as 9-tap shifted matmuls directly on the unpadded [128, 64, 64]
           image; border taps write clipped PSUM sub-regions (center tap first
           with start=True covers the full region; PSUM has_written handles the
           rest). lhsT = rw tiles [cin=128, cout=128] from scratch; 18 matmuls
           per PSUM group [cout=128, 8 rows x 64].
"""

from contextlib import ExitStack

import ml_dtypes
import numpy as np

import concourse.bass as bass
import concourse.tile as tile
from concourse import bacc, mybir
from concourse.bass_utils import run_bass_kernel_spmd

B, N, COUT, CIN, H, W = 16, 4, 256, 256, 64, 64
NCORES = 8
BPC = B // NCORES            # samples per core
NTAP = 9
KA = N * NTAP                # 36  stage-A contraction (n, q)
MA = BPC * NTAP              # 18  stage-A output rows (b_local, p)
MAP = 32                     # padded to one PE col-strip
M_TOT = CIN * COUT           # 65536
RPC = 8                      # output rows per PSUM group (N = 8*64 = 512)
NYC = H // RPC               # 8
CT = CIN // 128              # cin tiles (2)
OT = COUT // 128             # cout tiles (2)

BF16 = mybir.dt.bfloat16
F32 = mybir.dt.float32


def _build_graph():
    nc = bacc.Bacc(None, target_bir_lowering=False)

    x_d = nc.dram_tensor("x", [BPC, CIN, H, W], BF16, kind="ExternalInput")
    wf_d = nc.dram_tensor("wf", [KA, M_TOT], BF16, kind="ExternalInput")
    rmt_d = nc.dram_tensor("rmt", [KA, MAP], BF16, kind="ExternalInput")
    out_d = nc.dram_tensor("out", [BPC, COUT, H, W], F32, kind="ExternalOutput")

    with tile.TileContext(nc) as tc, ExitStack() as ctx:
        const_pool = ctx.enter_context(tc.tile_pool(name="const", bufs=1))
        wf_pool = ctx.enter_context(tc.tile_pool(name="wfin", bufs=3))
        rwst_pool = ctx.enter_context(tc.tile_pool(name="rwst", bufs=2))
        xin_pool = ctx.enter_context(tc.tile_pool(name="xin", bufs=1))
        rwt_pool = ctx.enter_context(tc.tile_pool(name="rwt", bufs=1))
        out_pool = ctx.enter_context(tc.tile_pool(name="outs", bufs=2))
        scr_pool = ctx.enter_context(tc.tile_pool(name="scr", bufs=1, space="DRAM"))

        # scratch viewed [18, 2 supergroups, 4 subtiles, 8192]
        scr = scr_pool.tile([MA, 2, 4, 8192], BF16)

        rmt_sb = const_pool.tile([KA, MAP], BF16)
        nc.sync.dma_start(rmt_sb[:], rmt_d[:])

        # input images: contiguous DMA into staging, gpsimd copy into
        # W-padded layout [128, 64, 66] (cols 0 and 65 zero)
        WP = W + 2
        xs = []
        for b in range(BPC):
            row = []
            for ct in range(CT):
                xst = xin_pool.tile([128, H, W], BF16, tag="xstage", bufs=2)
                nc.gpsimd.dma_start(xst[:], x_d[b, ct * 128 : (ct + 1) * 128])
                xt = xin_pool.tile([128, H, WP], BF16, tag=f"x{b}{ct}")
                nc.gpsimd.memset(xt[:, :, 0:1], 0.0)
                nc.gpsimd.memset(xt[:, :, W + 1 : WP], 0.0)
                nc.gpsimd.tensor_copy(xt[:, :, 1 : W + 1], xst[:])
                row.append(xt)
            xs.append(row)

        # ---- Stage A ----
        # tile t: chunks 16t .. 16t+15 ; chunk c=16t+4j+k -> col-group j, bank k
        NT = 8
        with tc.tile_pool(name="psa", bufs=2, space="PSUM") as psa_pool:
            rw_sb = None
            for t in range(NT):
                wf_sb = wf_pool.tile([KA, 16 * 512], BF16)
                nc.sync.dma_start(
                    wf_sb[:], wf_d[:, t * 8192 : (t + 1) * 8192]
                )
                ps = psa_pool.tile([128, 2048], F32)
                for k in range(4):
                    for j in range(4):
                        nc.tensor.matmul(
                            ps[32 * j : 32 * (j + 1), 512 * k : 512 * (k + 1)],
                            rmt_sb[:],
                            wf_sb[:, (4 * j + k) * 512 : (4 * j + k + 1) * 512],
                            start=True,
                            stop=True,
                            tile_position=(0, 32 * j),
                        )
                if t % 4 == 0:
                    rw_sb = rwst_pool.tile([128, 4, 2048], BF16, tag="rw")
                nc.vector.tensor_copy(rw_sb[:, t % 4, :], ps[:])
                if t % 4 == 3:
                    s = t // 4
                    for j in range(4):
                        nc.scalar.dma_start(
                            scr[:, s, :, 2048 * j : 2048 * (j + 1)],
                            rw_sb[32 * j : 32 * j + MA, :, :],
                        )

        # ---- Stage B ----
        # rw lhsT tiles [cin=128, (p, cout=128)] for each (b, ot, ct)
        scr_m = scr[:].rearrange("m s tt r -> m (s tt r)").rearrange(
            "m (c o) -> m c o", c=CIN
        )
        rwts = {}
        for b in range(BPC):
            for ot in range(OT):
                for ct in range(CT):
                    rwt = rwt_pool.tile([128, NTAP, 128], BF16, tag=f"rwt{b}{ot}{ct}")
                    src = scr_m[
                        b * NTAP : (b + 1) * NTAP,
                        ct * 128 : (ct + 1) * 128,
                        ot * 128 : (ot + 1) * 128,
                    ].rearrange("p c o -> c p o")
                    nc.scalar.dma_start(rwt[:], src)
                    rwts[(b, ot, ct)] = rwt

        TAPS = [(p, p // 3 - 1, p % 3 - 1) for p in [4, 0, 1, 2, 3, 5, 6, 7, 8]]
        with tc.tile_pool(name="psb", bufs=4, space="PSUM") as psb_pool:
            for b in range(BPC):
                for ot in range(OT):
                    for yc2 in range(NYC // 2):
                        ot_sb = out_pool.tile([128, 2 * RPC, W], F32)
                        for half in range(2):
                            yc = yc2 * 2 + half
                            y0 = yc * RPC
                            ps = psb_pool.tile([128, RPC, W], F32)
                            psf = ps[:].rearrange("m r c -> m (r c)")
                            nmm = 0
                            for ct in range(CT):
                                xt = xs[b][ct]
                                for p, dy, dx in TAPS:
                                    r0 = max(0, -(y0 + dy))
                                    r1 = RPC - max(0, y0 + RPC - 1 + dy - (H - 1))
                                    nmm += 1
                                    nc.tensor.matmul(
                                        psf[:, r0 * W : r1 * W],
                                        rwts[(b, ot, ct)][:, p, :],
                                        xt[:, y0 + dy + r0 : y0 + dy + r1, dx + 1 : dx + 1 + W],
                                        start=(nmm == 1),
                                        stop=(nmm == 2 * NTAP),
                                    )
                            nc.vector.tensor_copy(
                                ot_sb[:, half * RPC : (half + 1) * RPC, :], ps[:]
                            )
                        nc.gpsimd.dma_start(
                            out_d[
                                b,
                                ot * 128 : (ot + 1) * 128,
                                yc2 * 2 * RPC : (yc2 + 1) * 2 * RPC,
                            ],
                            ot_sb[:],
                        )

    nc.compile()
    return nc


_NC = None


def _get_nc():
    global _NC
    if _NC is None:
        _NC = _build_graph()
    return _NC


def _rot_mats_np(thetas):
    """thetas: [M] -> [M, 9, 9], numpy port of the reference builder."""
    thetas = np.asarray(thetas, np.float32)
    xc = np.cos(thetas)
    ys = np.sin(thetas)
    a = xc - ys
    b = xc * ys
    c = xc + ys
    z = np.zeros_like(xc)
    o = np.ones_like(xc)

    def mat(rows):
        return np.stack([np.stack(r, axis=-1) for r in rows], axis=-2)

    pos = mat([
        [a, 1 - a, z, z, z, z, z, z, z],
        [z, xc - b, b, z, 1 - c + b, ys - b, z, z, z],
        [z, z, a, z, z, 1 - a, z, z, z],
        [b, ys - b, z, xc - b, 1 - c + b, z, z, z, z],
        [z, z, z, z, o, z, z, z, z],
        [z, z, z, z, 1 - c + b, xc - b, z, ys - b, b],
        [z, z, z, 1 - a, z, z, a, z, z],
        [z, z, z, ys - b, 1 - c + b, z, b, xc - b, z],
        [z, z, z, z, z, z, z, 1 - a, a],
    ])
    neg = mat([
        [c, z, z, 1 - c, z, z, z, z, z],
        [-b, xc + b, z, b - ys, 1 - a - b, z, z, z, z],
        [z, 1 - c, c, z, z, z, z, z, z],
        [z, z, z, xc + b, 1 - a - b, z, -b, b - ys, z],
        [z, z, z, z, o, z, z, z, z],
        [z, b - ys, -b, z, 1 - a - b, xc + b, z, z, z],
        [z, z, z, z, z, z, c, 1 - c, z],
        [z, z, z, z, 1 - a - b, b - ys, z, xc + b, -b],
        [z, z, z, z, z, 1 - c, z, z, c],
    ])
    m = (thetas >= 0).astype(np.float32)[:, None, None]
    return m * pos + (1 - m) * neg


def _prep_inputs(x, alphas, angles, weight):
    x = np.asarray(x, np.float32)
    alphas = np.asarray(alphas, np.float32)
    angles = np.asarray(angles, np.float32)
    weight = np.asarray(weight, np.float32)

    rm = _rot_mats_np(angles.reshape(-1)).reshape(B, N, NTAP, NTAP)
    rm = rm * alphas[:, :, None, None]          # [b, n, p, q]
    # lhsT rows (n, q) = n*9+q ; cols (b, p) = b*9+p
    rmt = rm.transpose(1, 3, 0, 2).reshape(KA, B, NTAP)

    # wf rows (n, qy, qx) ; cols cin*COUT + cout
    wf = weight.transpose(0, 3, 4, 2, 1).reshape(KA, M_TOT)

    x_bf = x.astype(ml_dtypes.bfloat16)
    wf_bf = np.ascontiguousarray(wf).astype(ml_dtypes.bfloat16)

    in_maps = []
    for i in range(NCORES):
        rmt_i = np.zeros((KA, MAP), np.float32)
        rmt_i[:, :MA] = rmt[:, i * BPC : (i + 1) * BPC].reshape(KA, MA)
        in_maps.append({
            "x": np.ascontiguousarray(x_bf[i * BPC : (i + 1) * BPC]),
            "wf": wf_bf,
            "rmt": rmt_i.astype(ml_dtypes.bfloat16),
        })
    return in_maps


def _run(inputs, trace=False, **kw):
    nc = _get_nc()
    in_maps = _prep_inputs(**inputs)
    br = run_bass_kernel_spmd(nc, in_maps, core_ids=list(range(NCORES)),
                              trace=trace, **kw)
    out = np.concatenate([r["out"] for r in br.results], axis=0)
    return out, br


def kernel(x, alphas, angles, weight):
    out, _ = _run(dict(x=x, alphas=alphas, angles=angles, weight=weight))
    return out


if __name__ == "__main__":
    rng = np.random.default_rng(0)
    x = rng.standard_normal((B, CIN, H, W), np.float32)
    alphas = rng.random((B, N), np.float32)
    angles = (rng.standard_normal((B, N), np.float32) * 0.5).astype(np.float32)
    weight = rng.standard_normal((N, COUT, CIN, 3, 3), np.float32) * np.sqrt(
        2.0 / (COUT * 9)
    ).astype(np.float32)
    out = kernel(x=x, alphas=alphas, angles=angles, weight=weight)
    print(out.shape, out.dtype, np.abs(out).mean())


# revision 6
# speedup vs baseline: 1.3208x; 1.1956x over previous
"""AdaptiveRotatedConv2d on 8 TRN2 NeuronCores (data-parallel over batch).

Per core (2 samples):
  Stage A: rotated weights rw[b,p,cin,cout] = sum_{n,q} rm[b,n,p,q] * W[n,cout,cin,q]
           via TensorE matmuls: lhsT = rmt [36, 32] (stationary, cols 18..31
           zero), rhs = wf [36, m'] chunks (moving, m' = cin*COUT+cout).
           DMA-bandwidth tricks: wf tiles alternate between SBUF partition
           rows 0..35 and 64..99 (disjoint SDMA engine sets -> 2x rate) and
           between the two HWDGE rings (sync/scalar). Outputs are packed 4x
           into PSUM partition col-groups via tile_position; one DVE
           cast-copy per PSUM tile; bounced through DRAM scratch
           [b*9+p, cin, cout] (the bounce performs the (b,p)->cin transpose).
  Stage B: conv as 9-tap shifted matmuls on the host-W-padded [128, 64, 66]
           image; row-border taps write clipped (contiguous) PSUM
           sub-regions - the center tap goes first with start=True and full
           coverage, PSUM has_written handles partial accumulation.
           lhsT = rw tiles [cin=128, (p, cout=128)] from scratch; 18 matmuls
           per PSUM group [cout=128, 8 rows x 64].
"""

from contextlib import ExitStack

import ml_dtypes
import numpy as np

import concourse.bass as bass
import concourse.tile as tile
from concourse import bacc, mybir
from concourse.bass_utils import run_bass_kernel_spmd

B, N, COUT, CIN, H, W = 16, 4, 256, 256, 64, 64
NCORES = 8
BPC = B // NCORES            # samples per core
NTAP = 9
KA = N * NTAP                # 36  stage-A contraction (n, q)
MA = BPC * NTAP              # 18  stage-A output rows (b_local, p)
MAP = 32                     # padded to one PE col-strip
M_TOT = CIN * COUT           # 65536
RPC = 8                      # output rows per PSUM group (N = 8*64 = 512)
NYC = H // RPC               # 8
CT = CIN // 128              # cin tiles (2)
OT = COUT // 128             # cout tiles (2)
WP = W + 2                   # host-padded width

BF16 = mybir.dt.bfloat16
F32 = mybir.dt.float32


def _build_graph():
    nc = bacc.Bacc(None, target_bir_lowering=False)

    x_d = nc.dram_tensor("x", [BPC, CIN, H, WP], BF16, kind="ExternalInput")
    wf_d = nc.dram_tensor("wf", [KA, M_TOT], BF16, kind="ExternalInput")
    rmt_d = nc.dram_tensor("rmt", [128, MAP], BF16, kind="ExternalInput")
    out_d = nc.dram_tensor("out", [BPC, COUT, H, W], F32, kind="ExternalOutput")

    with tile.TileContext(nc) as tc, ExitStack() as ctx:
        const_pool = ctx.enter_context(tc.tile_pool(name="const", bufs=1))
        wf_pool = ctx.enter_context(tc.tile_pool(name="wfin", bufs=3))
        rwst_pool = ctx.enter_context(tc.tile_pool(name="rwst", bufs=2))
        xin_pool = ctx.enter_context(tc.tile_pool(name="xin", bufs=1))
        rwt_pool = ctx.enter_context(tc.tile_pool(name="rwt", bufs=1))
        out_pool = ctx.enter_context(tc.tile_pool(name="outs", bufs=2))
        scr_pool = ctx.enter_context(tc.tile_pool(name="scr", bufs=1, space="DRAM"))

        scr = scr_pool.tile([MA, M_TOT], BF16)

        # rmt rows 0..35 and 64..99 both hold the rotation-mix matrix
        rmt_sb = const_pool.tile([128, MAP], BF16)
        nc.sync.dma_start(rmt_sb[:], rmt_d[:])

        # input images (host-padded widths, contiguous loads on gpsimd queue)
        xs = []
        for b in range(BPC):
            row = []
            for ct in range(CT):
                xt = xin_pool.tile([128, H, WP], BF16, tag=f"x{b}{ct}")
                nc.gpsimd.dma_start(xt[:], x_d[b, ct * 128 : (ct + 1) * 128])
                row.append(xt)
            xs.append(row)

        # ---- Stage A ----
        # wf tile t (t=0..7): chunks 16t..16t+15, partition rows 0..35 (even t)
        # or 64..99 (odd t), ring sync (even) / scalar (odd).
        # psum tile t2 (0..15): chunks 8*t2..8*t2+7; chunk c = 8*t2 + 2*j + k
        # -> col-group j (partitions 32j..), bank k (free 512k..).
        # scr view for writes: col of chunk c, offset w = c*512+w with
        # c = 16P + 8a + 2j + k  ->  [m, P, a, j, k, w]
        scr_w = scr[:].rearrange(
            "m (P a j k w) -> m P a j k w", P=8, a=2, j=4, k=2, w=512
        )
        NT = 8
        with tc.tile_pool(name="psa", bufs=3, space="PSUM") as psa_pool:
            rw_sb = None
            for t2 in range(2 * NT):
                t = t2 // 2
                if t2 % 2 == 0:
                    wf_sb = wf_pool.tile([128, 16 * 512], BF16)
                    row0 = 64 * (t % 2)
                    eng = nc.sync if t % 2 == 0 else nc.scalar
                    eng.dma_start(
                        wf_sb[row0 : row0 + KA, :],
                        wf_d[:, t * 8192 : (t + 1) * 8192],
                    )
                a = t2 % 2
                row0 = 64 * (t % 2)
                ps = psa_pool.tile([128, 1024], F32)
                for j in range(4):
                    for k in range(2):
                        c_loc = (a * 8 + 2 * j + k) * 512
                        nc.tensor.matmul(
                            ps[32 * j : 32 * (j + 1), 512 * k : 512 * (k + 1)],
                            rmt_sb[row0 : row0 + KA, :],
                            wf_sb[row0 : row0 + KA, c_loc : c_loc + 512],
                            start=True,
                            stop=True,
                            tile_position=(row0, 32 * j),
                        )
                if a == 0:
                    rw_sb = rwst_pool.tile([128, 2, 2, 512], BF16, tag="rw")
                nc.vector.tensor_copy(
                    rw_sb[:].rearrange("m a k w -> m (a k w)")[
                        :, a * 1024 : (a + 1) * 1024
                    ],
                    ps[:],
                )
                if a == 1:
                    for j in range(4):
                        eng = nc.sync if j % 2 == 0 else nc.scalar
                        eng.dma_start(
                            scr_w[:, t, :, j, :, :],
                            rw_sb[32 * j : 32 * j + MA],
                        )

        # ---- Stage B ----
        scr_m = scr[:].rearrange("m (c o) -> m c o", c=CIN)
        rwts = {}
        for b in range(BPC):
            for ot in range(OT):
                for ct in range(CT):
                    rwt = rwt_pool.tile([128, NTAP, 128], BF16, tag=f"rwt{b}{ot}{ct}")
                    src = scr_m[
                        b * NTAP : (b + 1) * NTAP,
                        ct * 128 : (ct + 1) * 128,
                        ot * 128 : (ot + 1) * 128,
                    ].rearrange("p c o -> c p o")
                    nc.scalar.dma_start(rwt[:], src)
                    rwts[(b, ot, ct)] = rwt

        TAPS = [(p, p // 3 - 1, p % 3 - 1) for p in [4, 0, 1, 2, 3, 5, 6, 7, 8]]
        with tc.tile_pool(name="psb", bufs=4, space="PSUM") as psb_pool:
            for b in range(BPC):
                for ot in range(OT):
                    for yc2 in range(NYC // 2):
                        ot_sb = out_pool.tile([128, 2 * RPC, W], F32)
                        for half in range(2):
                            yc = yc2 * 2 + half
                            y0 = yc * RPC
                            ps = psb_pool.tile([128, RPC, W], F32)
                            psf = ps[:].rearrange("m r c -> m (r c)")
                            nmm = 0
                            for ct in range(CT):
                                xt = xs[b][ct]
                                for p, dy, dx in TAPS:
                                    r0 = max(0, -(y0 + dy))
                                    r1 = RPC - max(0, y0 + RPC - 1 + dy - (H - 1))
                                    nmm += 1
                                    nc.tensor.matmul(
                                        psf[:, r0 * W : r1 * W],
                                        rwts[(b, ot, ct)][:, p, :],
                                        xt[:, y0 + dy + r0 : y0 + dy + r1, dx + 1 : dx + 1 + W],
                                        start=(nmm == 1),
                                        stop=(nmm == 2 * NTAP),
                                    )
                            nc.vector.tensor_copy(
                                ot_sb[:, half * RPC : (half + 1) * RPC, :], ps[:]
                            )
                        nc.sync.dma_start(
                            out_d[
                                b,
                                ot * 128 : (ot + 1) * 128,
                                yc2 * 2 * RPC : (yc2 + 1) * 2 * RPC,
                            ],
                            ot_sb[:],
                        )

    nc.compile()
    return nc


_NC = None


def _get_nc():
    global _NC
    if _NC is None:
        _NC = _build_graph()
    return _NC


def _rot_mats_np(thetas):
    """thetas: [M] -> [M, 9, 9], numpy port of the reference builder."""
    thetas = np.asarray(thetas, np.float32)
    xc = np.cos(thetas)
    ys = np.sin(thetas)
    a = xc - ys
    b = xc * ys
    c = xc + ys
    z = np.zeros_like(xc)
    o = np.ones_like(xc)

    def mat(rows):
        return np.stack([np.stack(r, axis=-1) for r in rows], axis=-2)

    pos = mat([
        [a, 1 - a, z, z, z, z, z, z, z],
        [z, xc - b, b, z, 1 - c + b, ys - b, z, z, z],
        [z, z, a, z, z, 1 - a, z, z, z],
        [b, ys - b, z, xc - b, 1 - c + b, z, z, z, z],
        [z, z, z, z, o, z, z, z, z],
        [z, z, z, z, 1 - c + b, xc - b, z, ys - b, b],
        [z, z, z, 1 - a, z, z, a, z, z],
        [z, z, z, ys - b, 1 - c + b, z, b, xc - b, z],
        [z, z, z, z, z, z, z, 1 - a, a],
    ])
    neg = mat([
        [c, z, z, 1 - c, z, z, z, z, z],
        [-b, xc + b, z, b - ys, 1 - a - b, z, z, z, z],
        [z, 1 - c, c, z, z, z, z, z, z],
        [z, z, z, xc + b, 1 - a - b, z, -b, b - ys, z],
        [z, z, z, z, o, z, z, z, z],
        [z, b - ys, -b, z, 1 - a - b, xc + b, z, z, z],
        [z, z, z, z, z, z, c, 1 - c, z],
        [z, z, z, z, 1 - a - b, b - ys, z, xc + b, -b],
        [z, z, z, z, z, 1 - c, z, z, c],
    ])
    m = (thetas >= 0).astype(np.float32)[:, None, None]
    return m * pos + (1 - m) * neg


def _prep_inputs(x, alphas, angles, weight):
    x = np.asarray(x, np.float32)
    alphas = np.asarray(alphas, np.float32)
    angles = np.asarray(angles, np.float32)
    weight = np.asarray(weight, np.float32)

    rm = _rot_mats_np(angles.reshape(-1)).reshape(B, N, NTAP, NTAP)
    rm = rm * alphas[:, :, None, None]          # [b, n, p, q]
    # lhsT rows (n, q) = n*9+q ; cols (b, p) = b*9+p
    rmt = rm.transpose(1, 3, 0, 2).reshape(KA, B, NTAP)

    # wf rows (n, qy, qx) ; cols cin*COUT + cout
    wf = weight.transpose(0, 3, 4, 2, 1).reshape(KA, M_TOT)

    xpad = np.zeros((B, CIN, H, WP), np.float32)
    xpad[:, :, :, 1 : W + 1] = x
    x_bf = xpad.astype(ml_dtypes.bfloat16)
    wf_bf = np.ascontiguousarray(wf).astype(ml_dtypes.bfloat16)

    in_maps = []
    for i in range(NCORES):
        rmt_i = np.zeros((128, MAP), np.float32)
        rmt_i[:KA, :MA] = rmt[:, i * BPC : (i + 1) * BPC].reshape(KA, MA)
        rmt_i[64 : 64 + KA] = rmt_i[:KA]
        in_maps.append({
            "x": np.ascontiguousarray(x_bf[i * BPC : (i + 1) * BPC]),
            "wf": wf_bf,
            "rmt": rmt_i.astype(ml_dtypes.bfloat16),
        })
    return in_maps


def _run(inputs, trace=False, **kw):
    nc = _get_nc()
    in_maps = _prep_inputs(**inputs)
    br = run_bass_kernel_spmd(nc, in_maps, core_ids=list(range(NCORES)),
                              trace=trace, **kw)
    out = np.concatenate([r["out"] for r in br.results], axis=0)
    return out, br


def kernel(x, alphas, angles, weight):
    out, _ = _run(dict(x=x, alphas=alphas, angles=angles, weight=weight))
    return out


if __name__ == "__main__":
    rng = np.random.default_rng(0)
    x = rng.standard_normal((B, CIN, H, W), np.float32)
    alphas = rng.random((B, N), np.float32)
    angles = (rng.standard_normal((B, N), np.float32) * 0.5).astype(np.float32)
    weight = rng.standard_normal((N, COUT, CIN, 3, 3), np.float32) * np.sqrt(
        2.0 / (COUT * 9)
    ).astype(np.float32)
    out = kernel(x=x, alphas=alphas, angles=angles, weight=weight)
    print(out.shape, out.dtype, np.abs(out).mean())
